# revision 19
# baseline (speedup 1.0000x reference)
"""KernelNorm2d Trainium2 Bass kernel (fp16 I/O).

Problem: x [16, 64, 256, 256] f32. 2x2 windows (stride 2) over (H, W); per-window
statistics over (C, 2, 2) = 256 elements; out = (x - mean) / sqrt(var + eps).
Data-parallel over batch: 8 cores x 2 samples each.

Host converts to fp16 (end-to-end fp16 error ~3e-4 << 2e-2 tol), halving HBM
traffic. Per-core layout: partition = window-row i (nH = 128). SBUF tile
[128(i), C=64, a=2, W=256] fp16.

Measured engine facts driving the structure:
  - DVE tensor_reduce = 1 elem/cycle, period (no fast mode for any dtype/AP
    tried). The two reduction passes (sum, sum-sq) are DVE-only ops -> DVE
    carries ~136us/core minimum. Everything else is kept OFF the DVE.
  - Normalize (per-window scalars force per-j instructions): ACT ~550ns,
    GPSIMD ~760ns per 256-elem j-column -> split between those two engines.
  - ACT Square ~0.9ns/elem makes the squares.
  - Work is quartered into (sample, w-half) units; stats/normalize/store of a
    unit overlap the reduces of later units. Loads/stores are w-half sized
    (256B runs cost some DMA efficiency but start the pipeline earlier; DMA
    is far from critical here).
"""

import os
import sys

for _p in ("/opt/trn_rl_repo", "/root/.axon_site/_ro/trn_rl_repo"):
    if os.path.isdir(_p) and _p not in sys.path:
        sys.path.append(_p)

import numpy as np

import concourse.bass as bass
import concourse.tile as tile
from concourse import bacc, mybir
from concourse.bass_utils import run_bass_kernel_spmd

# Problem constants (hardcoded per spec nn_KernelNorm2d_72164040507639)
B, C, H, W = 16, 64, 256, 256
N_CORES = 8
B_LOC = B // N_CORES          # samples per core
NH = H // 2                   # 128 window rows = partition dim
NJ = W // 2                   # 128 window cols
NJH = NJ // 2                 # window cols per w-half
WH = W // 2
EPS = 1e-5
WIN = C * 4                   # 256 elements per window
CCH = 8                       # channels per square chunk

# normalize engine split per 64-j half: s=ACT, g=GPSIMD (DVE kept on reduces)
NS = 38                       # ACT share; rest -> GPSIMD


def _make_pattern(ns, n=64):
    w = {"s": ns, "g": n - ns}
    acc = {"s": 0.0, "g": 0.0}
    pat = []
    for k in range(n):
        best = max(w, key=lambda e: w[e] / n * (k + 1) - acc[e])
        acc[best] += 1
        pat.append(best)
    return "".join(pat)


NORM_PATTERN = _make_pattern(NS)


def build_kernel(debug: bool = False) -> bass.Bass:
    nc = bacc.Bacc("TRN2", debug=debug)
    f16 = mybir.dt.float16
    f32 = mybir.dt.float32
    x = nc.dram_tensor("x", [B_LOC, C, NH, 2, W], f16, kind="ExternalInput")
    y = nc.dram_tensor("y", [B_LOC, C, NH, 2, W], f16, kind="ExternalOutput")

    with tile.TileContext(nc) as tc:
        with (
            tc.tile_pool(name="data", bufs=2) as data_pool,
            tc.tile_pool(name="stats", bufs=2) as stats_pool,
            tc.tile_pool(name="scratch", bufs=2) as scratch_pool,
            tc.tile_pool(name="singles", bufs=1) as singles,
        ):
            eps_tile = singles.tile([NH, 1], f32)
            nc.vector.memset(eps_tile, EPS)

            state = {}

            def phase1(b, h, xt):
                """load half, window sums, sums of squares, stats."""
                ws = h * WH
                for a in range(2):
                    nc.sync.dma_start(
                        out=xt[:, :, a, ws : ws + WH],
                        in_=x[b, :, :, a, ws : ws + WH].transpose([1, 0, 2]),
                    )
                xh4 = xt[:, :, :, ws : ws + WH].rearrange(
                    "p c a (j b2) -> p j (c a) b2", b2=2
                )
                s_sum = stats_pool.tile([NH, NJH], f32, tag=f"s_sum{h}")
                nc.vector.tensor_reduce(
                    out=s_sum,
                    in_=xh4,
                    axis=mybir.AxisListType.XY,
                    op=mybir.AluOpType.add,
                )
                q_sum = stats_pool.tile([NH, NJH], f32, tag=f"q_sum{h}")
                q_part = stats_pool.tile([NH, NJH], f32, tag=f"q_part{h}")
                for ci in range(C // CCH):
                    cs = ci * CCH
                    x2 = scratch_pool.tile([NH, CCH, 2, WH], f16, tag=f"x2_{h}")
                    nc.scalar.activation(
                        out=x2,
                        in_=xt[:, cs : cs + CCH, :, ws : ws + WH],
                        func=mybir.ActivationFunctionType.Square,
                    )
                    x2v = x2.rearrange("p c a (j b2) -> p j (c a) b2", b2=2)
                    tgt = q_sum if ci == 0 else q_part
                    nc.vector.tensor_reduce(
                        out=tgt,
                        in_=x2v,
                        axis=mybir.AxisListType.XY,
                        op=mybir.AluOpType.add,
                    )
                    if ci > 0:
                        nc.vector.tensor_add(out=q_sum, in0=q_sum, in1=q_part)

                inv = stats_pool.tile([NH, NJH], f32, tag=f"inv{h}")
                tsh = stats_pool.tile([NH, NJH], f32, tag=f"tsh{h}")
                nm = stats_pool.tile([NH, NJH], f32, tag=f"nm{h}")
                var = stats_pool.tile([NH, NJH], f32, tag=f"var{h}")
                nm2 = stats_pool.tile([NH, NJH], f32, tag=f"nm2{h}")
                nc.vector.tensor_scalar_mul(out=nm, in0=s_sum, scalar1=-1.0 / WIN)
                nc.vector.tensor_mul(out=nm2, in0=nm, in1=nm)
                nc.vector.tensor_scalar_mul(out=var, in0=q_sum, scalar1=1.0 / WIN)
                nc.vector.tensor_tensor(
                    out=var, in0=var, in1=nm2, op=mybir.AluOpType.subtract
                )
                nc.scalar.activation(
                    out=var,
                    in_=var,
                    func=mybir.ActivationFunctionType.Sqrt,
                    bias=eps_tile,
                    scale=1.0,
                )
                nc.vector.reciprocal(out=inv, in_=var)
                nc.vector.tensor_mul(out=tsh, in0=nm, in1=inv)
                state[(b, h)] = (xt, inv, tsh)

            def phase2(b, h):
                """normalize half in place (ACT/GPSIMD) + store half."""
                xt, inv, tsh = state.pop((b, h))
                ws = h * WH
                xh4 = xt[:, :, :, ws : ws + WH].rearrange(
                    "p c a (j b2) -> p j (c a) b2", b2=2
                )
                for j in range(NJH):
                    win = xh4[:, j, :, :]
                    if NORM_PATTERN[j] == "s":
                        nc.scalar.activation(
                            out=win,
                            in_=win,
                            func=mybir.ActivationFunctionType.Identity,
                            bias=tsh[:, j : j + 1],
                            scale=inv[:, j : j + 1],
                        )
                    else:
                        nc.gpsimd.tensor_scalar(
                            out=win,
                            in0=win,
                            scalar1=inv[:, j : j + 1],
                            scalar2=tsh[:, j : j + 1],
                            op0=mybir.AluOpType.mult,
                            op1=mybir.AluOpType.add,
                        )
                for a in range(2):
                    nc.scalar.dma_start(
                        out=y[b, :, :, a, ws : ws + WH].transpose([1, 0, 2]),
                        in_=xt[:, :, a, ws : ws + WH],
                    )

            # software-pipelined emission over (sample, w-half) units:
            # phase1(00) p1(01) p2(00) p1(10) p2(01) p1(11) p2(10) p2(11)
            xt0 = data_pool.tile([NH, C, 2, W], f16, tag="xt")
            xt1 = data_pool.tile([NH, C, 2, W], f16, tag="xt")
            phase1(0, 0, xt0)
            phase1(0, 1, xt0)
            phase2(0, 0)
            phase1(1, 0, xt1)
            phase2(0, 1)
            phase1(1, 1, xt1)
            phase2(1, 0)
            phase2(1, 1)
    nc.compile()
    return nc


_NC_CACHE = None
LAST_RESULTS = None


def _get_nc():
    global _NC_CACHE
    if _NC_CACHE is None:
        _NC_CACHE = build_kernel()
    return _NC_CACHE


def kernel(x: np.ndarray) -> np.ndarray:
    global LAST_RESULTS
    assert x.shape == (B, C, H, W), x.shape
    xh = np.ascontiguousarray(x, dtype=np.float16).reshape(B, C, NH, 2, W)
    nc = _get_nc()
    in_maps = [{"x": xh[k * B_LOC : (k + 1) * B_LOC]} for k in range(N_CORES)]
    kw = {}
    if os.environ.get("KERNEL_TRACE") == "1":
        kw["trace"] = True
        if os.environ.get("KERNEL_TRACE_DIR"):
            kw["tmpdir"] = os.environ["KERNEL_TRACE_DIR"]
    res = run_bass_kernel_spmd(nc, in_maps, core_ids=list(range(N_CORES)), **kw)
    LAST_RESULTS = res
    out = np.concatenate([r["y"] for r in res.results], axis=0)
    return out.astype(np.float32).reshape(B, C, H, W)


# revision 22
# speedup vs baseline: 1.1103x; 1.1103x over previous
"""KernelNorm2d Trainium2 Bass kernel (fp16 I/O).

Problem: x [16, 64, 256, 256] f32. 2x2 windows (stride 2) over (H, W); per-window
statistics over (C, 2, 2) = 256 elements; out = (x - mean) / sqrt(var + eps).
Data-parallel over batch: 8 cores x 2 samples each.

Host converts to fp16 (end-to-end fp16 error ~3e-4 << 2e-2 tol), halving HBM
traffic. Per-core layout: partition = window-row i (nH = 128). SBUF tile
[128(i), C=64, a=2, W=256] fp16.

Measured engine facts driving the structure:
  - DVE tensor_reduce = 1 elem/cycle, period (no fast mode for any dtype/AP
    tried). The two reduction passes (sum, sum-sq) are DVE-only ops -> DVE
    carries ~136us/core minimum. Everything else is kept OFF the DVE.
  - Normalize (per-window scalars force per-j instructions): ACT ~550ns,
    GPSIMD ~760ns per 256-elem j-column -> split between those two engines.
  - ACT Square ~0.9ns/elem makes the squares.
  - Work is quartered into (sample, w-half) units; stats/normalize/store of a
    unit overlap the reduces of later units. Loads/stores are w-half sized
    (256B runs cost some DMA efficiency but start the pipeline earlier; DMA
    is far from critical here).
"""

import os
import sys

for _p in ("/opt/trn_rl_repo", "/root/.axon_site/_ro/trn_rl_repo"):
    if os.path.isdir(_p) and _p not in sys.path:
        sys.path.append(_p)

import numpy as np

import concourse.bass as bass
import concourse.tile as tile
from concourse import bacc, mybir
from concourse.bass_utils import run_bass_kernel_spmd

# Problem constants (hardcoded per spec nn_KernelNorm2d_72164040507639)
B, C, H, W = 16, 64, 256, 256
N_CORES = 8
B_LOC = B // N_CORES          # samples per core
NH = H // 2                   # 128 window rows = partition dim
NJ = W // 2                   # 128 window cols
NJH = NJ // 2                 # window cols per w-half
WH = W // 2
EPS = 1e-5
WIN = C * 4                   # 256 elements per window
CCH = 8                       # channels per square chunk

# normalize engine split per 64-j half (v=DVE, s=ACT, g=GPSIMD)
NV, NS = 8, 18                # DVE / ACT shares; rest -> GPSIMD


def _make_pattern(nv, ns, n=64):
    w = {"v": nv, "s": ns, "g": n - nv - ns}
    acc = {"v": 0.0, "s": 0.0, "g": 0.0}
    pat = []
    for k in range(n):
        best = max(w, key=lambda e: w[e] / n * (k + 1) - acc[e])
        acc[best] += 1
        pat.append(best)
    return "".join(pat)


NORM_PATTERN = _make_pattern(NV, NS)


def build_kernel(debug: bool = False) -> bass.Bass:
    nc = bacc.Bacc("TRN2", debug=debug)
    f16 = mybir.dt.float16
    f32 = mybir.dt.float32
    x = nc.dram_tensor("x", [B_LOC, C, NH, 2, W], f16, kind="ExternalInput")
    y = nc.dram_tensor("y", [B_LOC, C, NH, 2, W], f16, kind="ExternalOutput")

    with tile.TileContext(nc) as tc:
        with (
            tc.tile_pool(name="data", bufs=2) as data_pool,
            tc.tile_pool(name="stats", bufs=2) as stats_pool,
            tc.tile_pool(name="scratch", bufs=2) as scratch_pool,
            tc.tile_pool(name="singles", bufs=1) as singles,
        ):
            eps_tile = singles.tile([NH, 1], f32)
            nc.vector.memset(eps_tile, EPS)

            state = {}

            def load(b, xt):
                """two c-half DMAs per sample: (a w) merges -> 1 KiB runs."""
                for ch in range(2):
                    cs = ch * (C // 2)
                    nc.sync.dma_start(
                        out=xt[:, cs : cs + C // 2],
                        in_=x[b, cs : cs + C // 2].transpose([1, 0, 2, 3]),
                    )

            def phase1(b, h, xt):
                """window sums, sums of squares, stats for one w-half."""
                ws = h * WH
                xh4 = xt[:, :, :, ws : ws + WH].rearrange(
                    "p c a (j b2) -> p j (c a) b2", b2=2
                )
                s_sum = stats_pool.tile([NH, NJH], f32, tag=f"s_sum{h}")
                nc.vector.tensor_reduce(
                    out=s_sum,
                    in_=xh4,
                    axis=mybir.AxisListType.XY,
                    op=mybir.AluOpType.add,
                )
                q_sum = stats_pool.tile([NH, NJH], f32, tag=f"q_sum{h}")
                q_part = stats_pool.tile([NH, NJH], f32, tag=f"q_part{h}")
                for ci in range(C // CCH):
                    cs = ci * CCH
                    x2 = scratch_pool.tile([NH, CCH, 2, WH], f16, tag=f"x2_{h}")
                    nc.scalar.activation(
                        out=x2,
                        in_=xt[:, cs : cs + CCH, :, ws : ws + WH],
                        func=mybir.ActivationFunctionType.Square,
                    )
                    x2v = x2.rearrange("p c a (j b2) -> p j (c a) b2", b2=2)
                    tgt = q_sum if ci == 0 else q_part
                    nc.vector.tensor_reduce(
                        out=tgt,
                        in_=x2v,
                        axis=mybir.AxisListType.XY,
                        op=mybir.AluOpType.add,
                    )
                    if ci > 0:
                        nc.vector.tensor_add(out=q_sum, in0=q_sum, in1=q_part)

                inv = stats_pool.tile([NH, NJH], f32, tag=f"inv{h}")
                tsh = stats_pool.tile([NH, NJH], f32, tag=f"tsh{h}")
                nm = stats_pool.tile([NH, NJH], f32, tag=f"nm{h}")
                var = stats_pool.tile([NH, NJH], f32, tag=f"var{h}")
                nm2 = stats_pool.tile([NH, NJH], f32, tag=f"nm2{h}")
                nc.vector.tensor_scalar_mul(out=nm, in0=s_sum, scalar1=-1.0 / WIN)
                nc.vector.tensor_mul(out=nm2, in0=nm, in1=nm)
                nc.vector.tensor_scalar_mul(out=var, in0=q_sum, scalar1=1.0 / WIN)
                nc.vector.tensor_tensor(
                    out=var, in0=var, in1=nm2, op=mybir.AluOpType.subtract
                )
                nc.scalar.activation(
                    out=var,
                    in_=var,
                    func=mybir.ActivationFunctionType.Sqrt,
                    bias=eps_tile,
                    scale=1.0,
                )
                nc.vector.reciprocal(out=inv, in_=var)
                nc.vector.tensor_mul(out=tsh, in0=nm, in1=inv)
                state[(b, h)] = (xt, inv, tsh)

            def phase2(b, h):
                """normalize half in place (DVE/ACT/GPSIMD split)."""
                xt, inv, tsh = state.pop((b, h))
                ws = h * WH
                xh4 = xt[:, :, :, ws : ws + WH].rearrange(
                    "p c a (j b2) -> p j (c a) b2", b2=2
                )
                for j in range(NJH):
                    win = xh4[:, j, :, :]
                    eng = NORM_PATTERN[j]
                    if eng == "s":
                        nc.scalar.activation(
                            out=win,
                            in_=win,
                            func=mybir.ActivationFunctionType.Identity,
                            bias=tsh[:, j : j + 1],
                            scale=inv[:, j : j + 1],
                        )
                    else:
                        e = nc.vector if eng == "v" else nc.gpsimd
                        e.tensor_scalar(
                            out=win,
                            in0=win,
                            scalar1=inv[:, j : j + 1],
                            scalar2=tsh[:, j : j + 1],
                            op0=mybir.AluOpType.mult,
                            op1=mybir.AluOpType.add,
                        )
                if h == 1:
                    nc.scalar.dma_start(
                        out=y[b].transpose([1, 0, 2, 3]), in_=xt
                    )

            # software-pipelined emission over (sample, w-half) units
            xt0 = data_pool.tile([NH, C, 2, W], f16, tag="xt")
            xt1 = data_pool.tile([NH, C, 2, W], f16, tag="xt")
            load(0, xt0)
            phase1(0, 0, xt0)
            phase1(0, 1, xt0)
            phase2(0, 0)
            load(1, xt1)
            phase1(1, 0, xt1)
            phase2(0, 1)
            phase1(1, 1, xt1)
            phase2(1, 0)
            phase2(1, 1)
    nc.compile()
    return nc


_NC_CACHE = None
LAST_RESULTS = None


def _get_nc():
    global _NC_CACHE
    if _NC_CACHE is None:
        _NC_CACHE = build_kernel()
    return _NC_CACHE


def kernel(x: np.ndarray) -> np.ndarray:
    global LAST_RESULTS
    assert x.shape == (B, C, H, W), x.shape
    xh = np.ascontiguousarray(x, dtype=np.float16).reshape(B, C, NH, 2, W)
    nc = _get_nc()
    in_maps = [{"x": xh[k * B_LOC : (k + 1) * B_LOC]} for k in range(N_CORES)]
    kw = {}
    if os.environ.get("KERNEL_TRACE") == "1":
        kw["trace"] = True
        if os.environ.get("KERNEL_TRACE_DIR"):
            kw["tmpdir"] = os.environ["KERNEL_TRACE_DIR"]
    res = run_bass_kernel_spmd(nc, in_maps, core_ids=list(range(N_CORES)), **kw)
    LAST_RESULTS = res
    out = np.concatenate([r["y"] for r in res.results], axis=0)
    return out.astype(np.float32).reshape(B, C, H, W)
